# revision 37
# baseline (speedup 1.0000x reference)
"""Trainium2 Bass kernel for CrossLevelAttention (L=4, N=65536, D=512).

Strategy (8 NeuronCores, SPMD):
  - Data-parallel shard of the node dim N (8192 nodes/core/level).
  - Pass 1 reads a host-precast bf16 copy of x (halves pass-1 HBM
    traffic; numerically identical to the previous on-chip cast):
    two DVE pairwise-add rounds quarter the PE ones-matvec work, then
    PE accumulates per-level feature sums in PSUM.
  - Warmup AllReduce at t=0 absorbs collective cold-start/skew.
  - AllReduce #1 (8 KiB) -> replicated level summaries.
  - Attention/MLP weights are E-sharded across cores (each core holds a
    64-wide output slice of Wq/Wk/Wv/W1 and the matching 64 rows of W2):
      q/k/v partial projections -> AllGather (24 KiB) -> full q,k,v
      -> scores/softmax/ctx replicated -> per-core MLP slice
      -> AllReduce #2 (8 KiB) -> replicated residual update `upd`
      broadcast across partitions via a rank-1 PE matmul.
  - Pass 2 streams fp32 x tiles [128, 8, 512] (16 KiB/partition
    descriptors); fused residual-add + LayerNorm split across DVE and
    ACT engines. Loads ride the sync queue (kept free of any
    collective-dependent instruction so prefetch fills the collective
    window); stores ride the vector queue.
"""

import os
import sys

import numpy as np

for _p in ("/opt/trn_rl_repo", "/root/.axon_site/_ro/trn_rl_repo"):
    if os.path.isdir(_p) and _p not in sys.path:
        sys.path.append(_p)

import ml_dtypes

import concourse.bass as bass
import concourse.bacc as bacc
import concourse.mybir as mybir
import concourse.tile as tile
from concourse.bass_utils import run_bass_kernel_spmd

L = 4
N_FULL = 65536
D = 512
NUM_CORES = 8
P = 128                # SBUF partitions
G1 = 16                # bf16 nodes/partition in a pass-1 tile (16 KiB/part)
G2 = 8                 # fp32 nodes/partition in a pass-2 tile (16 KiB/part)
CH = D // P            # 4 feature chunks of 128
ES = D // NUM_CORES    # 64-wide E-shard per core
NEG_INF = -1e30
SCALE = (D // 4) ** -0.5
LN_EPS = 1e-5
BB = 7                 # pass-2 tile pool depth (prefetch window)

F32 = mybir.dt.float32
BF16 = mybir.dt.bfloat16
ALU = mybir.AluOpType
ACT = mybir.ActivationFunctionType
AX = mybir.AxisListType


def build(n_per_core: int, trivial_affine: bool, num_devices: int = NUM_CORES):
    """Build the SPMD Bass program for one core."""
    assert n_per_core % (P * G1) == 0 and n_per_core % (P * G2) == 0
    T1 = n_per_core // (P * G1)        # pass-1 tiles per level
    T2 = n_per_core // (P * G2)        # pass-2 tiles per level
    n_total = n_per_core * num_devices
    rg = [list(range(num_devices))]

    nc = bacc.Bacc(
        "TRN2", target_bir_lowering=False, debug=False, num_devices=num_devices
    )

    x16_d = nc.dram_tensor("x16", [L, n_per_core, D], BF16, kind="ExternalInput")
    x_d = nc.dram_tensor("x", [L, n_per_core, D], F32, kind="ExternalInput")
    # weights pre-packed on host into the exact SBUF layouts (contiguous DMA)
    wq_d = nc.dram_tensor("wq", [P, L * CH * ES], F32, kind="ExternalInput")
    wk_d = nc.dram_tensor("wk", [P, L * CH * ES], F32, kind="ExternalInput")
    wv_d = nc.dram_tensor("wv", [P, L * CH * ES], F32, kind="ExternalInput")
    w1_d = nc.dram_tensor("w1", [P, L * CH * ES], F32, kind="ExternalInput")
    w2_d = nc.dram_tensor("w2", [ES, L * CH * P], F32, kind="ExternalInput")
    bqk_d = nc.dram_tensor("bqk", [P, 2 * CH * L], F32, kind="ExternalInput")
    bv_d = nc.dram_tensor("bv", [L, D], F32, kind="ExternalInput")
    b1c_d = nc.dram_tensor("b1c", [ES, L], F32, kind="ExternalInput")
    # b2/num_devices in column layout [p, (l, c)]; folded into the
    # pre-AllReduce partial so the sum over cores adds exactly b2
    b2c_d = nc.dram_tensor("b2c", [P, L * CH], F32, kind="ExternalInput")
    eye_d = nc.dram_tensor("eye4", [L, L], F32, kind="ExternalInput")
    mask_d = nc.dram_tensor("maskdiv", [L, L], F32, kind="ExternalInput")
    if not trivial_affine:
        gam_d = nc.dram_tensor("gamma", [1, L * D], F32, kind="ExternalInput")
        bet_d = nc.dram_tensor("beta", [1, L * D], F32, kind="ExternalInput")
    # bf16 output: LayerNorm-output rounding is purely multiplicative
    # (<= 2^-9 relative); the host upcasts to fp32 after gathering
    out_d = nc.dram_tensor("out", [L, n_per_core, D], BF16, kind="ExternalOutput")

    x16_r = x16_d.ap().rearrange("l (t p g) d -> l t p g d", p=P, g=G1)
    x_r = x_d.ap().rearrange("l (t p g) d -> l t p g d", p=P, g=G2)
    out_r = out_d.ap().rearrange("l (t p g) d -> l t p g d", p=P, g=G2)

    with tile.TileContext(nc) as tc:
        with (
            tc.tile_pool(name="const", bufs=1) as cpool,
            tc.tile_pool(name="wpool", bufs=1) as wpool,
            tc.tile_pool(name="xb", bufs=BB) as xpool,
            tc.tile_pool(name="ob", bufs=3) as opool,
            tc.tile_pool(name="srp", bufs=2) as srpool,
            tc.tile_pool(name="scr", bufs=2) as scrpool,
            tc.tile_pool(name="stats", bufs=4) as stpool,
            tc.tile_pool(name="small", bufs=1) as spool,
            tc.tile_pool(name="psA", bufs=1, space="PSUM") as psA,
            tc.tile_pool(name="dram", bufs=1, space="DRAM") as dram,
        ):
            ones16 = cpool.tile([P, 1], BF16)
            nc.vector.memset(ones16[:], 1.0)
            onesr = cpool.tile([1, P], F32)
            nc.vector.memset(onesr[:], 1.0)
            eps_sb = cpool.tile([P, 1], F32)
            nc.vector.memset(eps_sb[:], LN_EPS)

            # ---------------- warmup collective (absorb cold-start) ----------
            warm = spool.tile([1, 8], F32)
            nc.vector.memset(warm[:], 0.0)
            war_in = dram.tile([1, 8], F32)
            war_out = dram.tile([1, 8], F32)
            nc.gpsimd.dma_start(war_in[:], warm[:])
            nc.gpsimd.collective_compute(
                "AllReduce", ALU.add, replica_groups=rg,
                ins=[war_in.opt()], outs=[war_out.opt()],
            )

            # ---------------- small consts (tiny, ahead of the streams) ------
            bqk_sb = cpool.tile([P, 2, CH, L], F32)
            bv_sb = cpool.tile([L, D], F32)
            b1c_sb = cpool.tile([ES, L], F32)
            b2c_sb = cpool.tile([P, L, CH], F32)
            eye_sb = cpool.tile([L, L], F32)
            mask_sb = cpool.tile([L, L], F32)
            nc.sync.dma_start(
                bqk_sb[:], bqk_d.ap().rearrange("p (b c l) -> p b c l", b=2, c=CH)
            )
            nc.sync.dma_start(
                b2c_sb[:], b2c_d.ap().rearrange("p (l c) -> p l c", l=L)
            )
            for sb, dt_ in (
                (bv_sb, bv_d), (b1c_sb, b1c_d),
                (eye_sb, eye_d), (mask_sb, mask_d),
            ):
                nc.sync.dma_start(sb[:], dt_.ap())

            # ---------------- Pass 1: partial sums over this core's nodes ----
            psum_rows = [
                psA.tile([1, D], F32, tag=f"p{i}", name=f"prow{i}")
                for i in range(2)
            ]
            # one AllReduce per level, fired as soon as that level's sum is
            # ready -- all but the last ride under the pass-1 DMA stream
            ar1_ins = [
                dram.tile([1, D], F32, name=f"ar1i{lv}") for lv in range(L)
            ]
            ar1_outs = [
                dram.tile([1, D], F32, name=f"ar1o{lv}") for lv in range(L)
            ]
            for lv in range(L):
                pr = psum_rows[lv % 2]
                for t in range(T1):
                    xb16 = xpool.tile([P, G1, D], BF16, tag="xb", name="xb16")
                    nc.sync.dma_start(xb16[:], x16_r[lv, t])
                    # two in-place pairwise-add rounds quarter the PE work
                    nc.vector.tensor_add(
                        xb16[:, : G1 // 2, :],
                        xb16[:, : G1 // 2, :], xb16[:, G1 // 2 :, :],
                    )
                    nc.vector.tensor_add(
                        xb16[:, : G1 // 4, :],
                        xb16[:, : G1 // 4, :], xb16[:, G1 // 4 : G1 // 2, :],
                    )
                    for g in range(G1 // 4):
                        nc.tensor.matmul(
                            pr[:],
                            lhsT=ones16[:],
                            rhs=xb16[:, g, :],
                            start=(t == 0 and g == 0),
                            stop=(t == T1 - 1 and g == G1 // 4 - 1),
                        )
                srow = srpool.tile([1, D], F32, tag="srow", name="srow")
                nc.vector.tensor_scalar_mul(srow[:], pr[:], 1.0 / n_total)
                nc.gpsimd.dma_start(ar1_ins[lv][:], srow[:])
                nc.gpsimd.collective_compute(
                    "AllReduce", ALU.add, replica_groups=rg,
                    ins=[ar1_ins[lv].opt()], outs=[ar1_outs[lv].opt()],
                )

            # ---------------- weights (contiguous descriptors) ---------------
            wq_sb = wpool.tile([P, L, CH, ES], F32)
            wk_sb = wpool.tile([P, L, CH, ES], F32)
            wv_sb = wpool.tile([P, L, CH, ES], F32)
            w1_sb = wpool.tile([P, L, CH, ES], F32)
            for wsb, wd in ((wq_sb, wq_d), (wk_sb, wk_d), (wv_sb, wv_d), (w1_sb, w1_d)):
                nc.sync.dma_start(
                    wsb[:], wd.ap().rearrange("p (l c e) -> p l c e", l=L, c=CH)
                )
            w2_sb = wpool.tile([ES, L, CH, P], F32)
            nc.sync.dma_start(
                w2_sb[:], w2_d.ap().rearrange("m (l c p) -> m l c p", l=L, c=CH)
            )

            # ---------------- pass-2 prefetch (fills the collective window) --
            p2_tiles = []
            for i in range(BB):
                lv, t = divmod(i, T2)
                xb = xpool.tile([P, G2, D], F32, tag="xb", name="xbp")
                nc.sync.dma_start(xb[:], x_r[lv, t])
                p2_tiles.append(xb)

            # ------- per-level: summary readback, transpose, q/k/v -----------
            summ_col = spool.tile([P, CH, L], F32)
            psum_sc = psA.tile([P, CH, L], F32, tag="big", name="psum_sc")
            psum_qkv = psA.tile([ES, 3, L], F32, tag="h", name="psum_qkv")
            for lv in range(L):
                srow_lv = spool.tile([1, D], F32, name=f"summr{lv}")
                nc.gpsimd.dma_start(srow_lv[:], ar1_outs[lv][:])
                for c in range(CH):
                    nc.tensor.matmul(
                        psum_sc[:, c, lv : lv + 1],
                        lhsT=srow_lv[:, bass.ts(c, P)],
                        rhs=onesr[:, 0:1],
                        is_transpose=True,
                        start=True,
                        stop=True,
                    )
                nc.vector.tensor_copy(
                    summ_col[:, :, lv : lv + 1], psum_sc[:, :, lv : lv + 1]
                )
                for ti, wsb in enumerate((wq_sb, wk_sb, wv_sb)):
                    for c in range(CH):
                        nc.tensor.matmul(
                            psum_qkv[:, ti, lv : lv + 1],
                            lhsT=wsb[:, lv, c, :],
                            rhs=summ_col[:, c, lv : lv + 1],
                            start=(c == 0),
                            stop=(c == CH - 1),
                        )
            qkv_sb = spool.tile([ES, 3, L], F32)
            nc.vector.tensor_copy(qkv_sb[:], psum_qkv[:])

            ag_in = dram.tile([ES, 3 * L], F32)
            ag_out = dram.tile([ES * num_devices, 3 * L], F32)
            nc.gpsimd.dma_start(ag_in[:], qkv_sb[:])
            nc.gpsimd.collective_compute(
                "AllGather", ALU.bypass, replica_groups=rg,
                ins=[ag_in.opt()], outs=[ag_out.opt()],
            )

            # ag_out rows = global e index (rank-major), cols = (tensor, level)
            # loads ride the scalar queue: the sync queue must stay free of
            # collective-dependent instructions so pass-2 prefetch flows.
            ag_r = ag_out[:].rearrange("(c p) (t l) -> t p c l", p=P, l=L)
            q_col = spool.tile([P, CH, L], F32)
            k_col = spool.tile([P, CH, L], F32)
            nc.scalar.dma_start(q_col[:], ag_r[0])
            nc.scalar.dma_start(k_col[:], ag_r[1])
            v_row = spool.tile([L, D], F32)
            nc.scalar.dma_start(
                v_row[:], ag_out[:].rearrange("e (t l) -> t l e", l=L)[2]
            )

            nc.vector.tensor_add(q_col[:], q_col[:], bqk_sb[:, 0])
            nc.vector.tensor_add(k_col[:], k_col[:], bqk_sb[:, 1])
            nc.vector.tensor_add(v_row[:], v_row[:], bv_sb[:])

            # ---------------- scores / masked softmax ------------------------
            psum_s = psA.tile([L, L], F32, tag="u", name="psum_s")
            for c in range(CH):
                nc.tensor.matmul(
                    psum_s[:],
                    lhsT=q_col[:, c, :],
                    rhs=k_col[:, c, :],
                    start=(c == 0),
                    stop=(c == CH - 1),
                )
            s_sb = spool.tile([L, L], F32)
            nc.vector.tensor_add(s_sb[:], psum_s[:], mask_sb[:])
            probs = spool.tile([L, L], F32)
            nc.scalar.activation(probs[:], s_sb[:], ACT.Exp, scale=SCALE)
            rs = spool.tile([L, 1], F32)
            nc.vector.tensor_reduce(rs[:], probs[:], axis=AX.X, op=ALU.add)
            rcp = spool.tile([L, 1], F32)
            nc.vector.reciprocal(rcp[:], rs[:])
            pn = spool.tile([L, L], F32)
            nc.vector.tensor_scalar_mul(pn[:], probs[:], rcp[:])

            psum_pT = psA.tile([L, L], F32, tag="p0", name="psum_pT")
            nc.tensor.transpose(psum_pT[:], pn[:], eye_sb[:])
            pT = spool.tile([L, L], F32)
            nc.vector.tensor_copy(pT[:], psum_pT[:])

            # ---------------- ctx (column layout), per-core MLP slice --------
            psum_ctx = psA.tile([P, CH, L], F32, tag="p1", name="psum_ctx")
            for c in range(CH):
                nc.tensor.matmul(
                    psum_ctx[:, c, :],
                    lhsT=v_row[:, bass.ts(c, P)],
                    rhs=pT[:],
                    start=(c == 0),
                    stop=(c == CH - 1),
                )
            ctx_col = spool.tile([P, CH, L], F32)
            nc.vector.tensor_copy(ctx_col[:], psum_ctx[:])

            psum_h = psA.tile([ES, L], F32, tag="h", name="psum_h")
            for lv in range(L):
                for c in range(CH):
                    nc.tensor.matmul(
                        psum_h[:, lv : lv + 1],
                        lhsT=w1_sb[:, lv, c, :],
                        rhs=ctx_col[:, c, lv : lv + 1],
                        start=(lv == 0 and c == 0),
                        stop=(lv == L - 1 and c == CH - 1),
                    )
            h_sb = spool.tile([ES, L], F32)
            nc.vector.scalar_tensor_tensor(
                h_sb[:], psum_h[:], 1.0, b1c_sb[:], ALU.mult, ALU.add
            )
            nc.vector.tensor_relu(h_sb[:], h_sb[:])

            # upd partial in column layout: upd[e=c*128+p, l]
            psum_u = psA.tile([P, L, CH], F32, tag="u", name="psum_u")
            for lv in range(L):
                for c in range(CH):
                    nc.tensor.matmul(
                        psum_u[:, lv, c : c + 1],
                        lhsT=w2_sb[:, lv, c, :],
                        rhs=h_sb[:, lv : lv + 1],
                        start=(lv == 0 and c == 0),
                        stop=(lv == L - 1 and c == CH - 1),
                    )
            up_sb = spool.tile([P, L, CH], F32)
            nc.vector.scalar_tensor_tensor(
                up_sb[:], psum_u[:], 1.0, b2c_sb[:], ALU.mult, ALU.add
            )

            ar3_in = dram.tile([P, L * CH], F32)
            ar3_out = dram.tile([P, L * CH], F32)
            nc.gpsimd.dma_start(ar3_in[:], up_sb[:])
            nc.gpsimd.collective_compute(
                "AllReduce", ALU.add, replica_groups=rg,
                ins=[ar3_in.opt()], outs=[ar3_out.opt()],
            )
            # read back on one partition: flattened free axis of us_sb is
            # the feature index d = c*128 + p per level (b2 already summed in)
            us_sb = spool.tile([1, L, CH, P], F32)
            ar3_r = ar3_out[:].rearrange("p (l c) -> l c p", l=L)
            for lv in range(L):
                nc.gpsimd.dma_start(us_sb[:, lv], ar3_r[lv])
            upd_row = us_sb[:].rearrange("o l c p -> o (l c p)")

            # broadcast upd across partitions with a rank-1 PE matmul;
            # per-level so pass-2 level 0 can start after the first copy
            psum_ubc = psA.tile([P, L, D], F32, tag="big", name="psum_ubc")
            upd_bc = cpool.tile([P, L, D], F32)
            for lv in range(L):
                nc.tensor.matmul(
                    psum_ubc[:, lv, :],
                    lhsT=onesr[:],
                    rhs=upd_row[:, bass.ts(lv, D)],
                    start=True,
                    stop=True,
                )
                nc.vector.tensor_copy(upd_bc[:, lv, :], psum_ubc[:, lv, :])

            if not trivial_affine:
                gam_row = spool.tile([1, L * D], F32)
                bet_row = spool.tile([1, L * D], F32)
                nc.scalar.dma_start(gam_row[:], gam_d.ap())
                nc.scalar.dma_start(bet_row[:], bet_d.ap())
                psum_gbc = psA.tile([P, L, D], F32, tag="big", name="psum_gbc")
                gam_bc = cpool.tile([P, L, D], F32)
                bet_bc = cpool.tile([P, L, D], F32)
                for lv in range(L):
                    nc.tensor.matmul(
                        psum_gbc[:, lv, :], lhsT=onesr[:],
                        rhs=gam_row[:, bass.ts(lv, D)], start=True, stop=True,
                    )
                nc.scalar.activation(gam_bc[:], psum_gbc[:], ACT.Identity)
                for lv in range(L):
                    nc.tensor.matmul(
                        psum_gbc[:, lv, :], lhsT=onesr[:],
                        rhs=bet_row[:, bass.ts(lv, D)], start=True, stop=True,
                    )
                nc.scalar.activation(bet_bc[:], psum_gbc[:], ACT.Identity)

            # ---------------- Pass 2: residual + LayerNorm -------------------
            NSC = 1  # normalize groups per half-tile handled by scalar
            idx = 0
            for lv in range(L):
                for t in range(T2):
                    if idx < BB:
                        xb = p2_tiles[idx]
                    else:
                        xb = xpool.tile([P, G2, D], F32, tag="xb", name="xb")
                        nc.sync.dma_start(xb[:], x_r[lv, t])
                    idx += 1

                    ob = opool.tile([P, G2, D], BF16, tag="ob")
                    GH = G2 // 2
                    # two half-tile LN pipelines: per-half stats and stores
                    # halve the restart latency and the drain
                    for h in range(2):
                        g0 = h * GH
                        sums = stpool.tile([P, GH], F32, tag=f"sums{h}")
                        ssq = stpool.tile([P, GH], F32, tag=f"ssq{h}")
                        for g in range(g0, g0 + GH):
                            nc.vector.scalar_tensor_tensor(
                                xb[:, g, :], xb[:, g, :], 1.0, upd_bc[:, lv, :],
                                ALU.mult, ALU.add,
                                accum_out=sums[:, g - g0 : g - g0 + 1],
                            )
                        scr = scrpool.tile([P, D], F32, tag="scr")
                        for g in range(g0, g0 + GH):
                            nc.scalar.activation(
                                scr[:], xb[:, g, :], ACT.Square,
                                accum_out=ssq[:, g - g0 : g - g0 + 1],
                            )
                        mu = stpool.tile([P, GH], F32, tag=f"mu{h}")
                        nc.vector.tensor_scalar_mul(mu[:], sums[:], 1.0 / D)
                        msq = stpool.tile([P, GH], F32, tag=f"msq{h}")
                        nc.vector.tensor_mul(msq[:], mu[:], mu[:])
                        var = stpool.tile([P, GH], F32, tag=f"var{h}")
                        nc.vector.scalar_tensor_tensor(
                            var[:], ssq[:], 1.0 / D, msq[:],
                            ALU.mult, ALU.subtract,
                        )
                        std = stpool.tile([P, GH], F32, tag=f"std{h}")
                        nc.scalar.activation(
                            std[:], var[:], ACT.Sqrt, bias=eps_sb[:]
                        )
                        inv = stpool.tile([P, GH], F32, tag=f"inv{h}")
                        nc.vector.reciprocal(inv[:], std[:])
                        nmi = stpool.tile([P, GH], F32, tag=f"nmi{h}")
                        nc.vector.scalar_tensor_tensor(
                            nmi[:], mu[:], -1.0, inv[:], ALU.mult, ALU.mult
                        )

                        for g in range(g0, g0 + GH):
                            gl = g - g0
                            if trivial_affine:
                                if gl < NSC:
                                    nc.scalar.activation(
                                        ob[:, g, :], xb[:, g, :], ACT.Identity,
                                        bias=nmi[:, gl : gl + 1],
                                        scale=inv[:, gl : gl + 1],
                                    )
                                else:
                                    nc.vector.tensor_scalar(
                                        ob[:, g, :], xb[:, g, :],
                                        inv[:, gl : gl + 1],
                                        nmi[:, gl : gl + 1],
                                        ALU.mult, ALU.add,
                                    )
                            else:
                                if gl < NSC:
                                    nc.scalar.activation(
                                        xb[:, g, :], xb[:, g, :], ACT.Identity,
                                        bias=nmi[:, gl : gl + 1],
                                        scale=inv[:, gl : gl + 1],
                                    )
                                else:
                                    nc.vector.tensor_scalar(
                                        xb[:, g, :], xb[:, g, :],
                                        inv[:, gl : gl + 1],
                                        nmi[:, gl : gl + 1],
                                        ALU.mult, ALU.add,
                                    )
                                nc.vector.tensor_mul(
                                    xb[:, g, :], xb[:, g, :], gam_bc[:, lv, :]
                                )
                                nc.gpsimd.tensor_tensor(
                                    ob[:, g, :], xb[:, g, :], bet_bc[:, lv, :],
                                    op=ALU.add,
                                )
                        nc.gpsimd.dma_start(
                            out_r[lv, t][:, g0 : g0 + GH, :],
                            ob[:, g0 : g0 + GH, :],
                        )

    nc.compile()
    return nc


def make_in_maps(inputs: dict, n_per_core: int, trivial_affine: bool,
                 num_devices: int = NUM_CORES):
    """Shard full inputs into per-core input maps."""
    f = lambda a: np.ascontiguousarray(np.asarray(a, dtype=np.float32))
    x = f(inputs["x"])
    x16 = x.astype(ml_dtypes.bfloat16)
    Wq, Wk, Wv = f(inputs["Wq"]), f(inputs["Wk"]), f(inputs["Wv"])
    W1, W2 = f(inputs["W1"]), f(inputs["W2"])
    bq, bk, bv = f(inputs["bq"]), f(inputs["bk"]), f(inputs["bv"])
    b1, b2 = f(inputs["b1"]), f(inputs["b2"])

    es = D // num_devices

    def pack_w(W, es_sl):
        # [l, d, e] -> [p, (l, c, e)] with d = c*128 + p
        w = W[:, :, es_sl].reshape(L, CH, P, es)
        return np.ascontiguousarray(w.transpose(2, 0, 1, 3).reshape(P, -1))

    def pack_w2(W, es_sl):
        # [l, e, d] -> [e, (l, c, p)] with d = c*128 + p
        w = W[:, es_sl, :].reshape(L, es, CH, P)
        return np.ascontiguousarray(w.transpose(1, 0, 2, 3).reshape(es, -1))

    # bias in column layout [p, c, l] with d = c*128 + p; q and k packed
    bq_col = bq.reshape(L, CH, P).transpose(2, 1, 0)
    bk_col = bk.reshape(L, CH, P).transpose(2, 1, 0)
    bqk = np.ascontiguousarray(
        np.stack([bq_col, bk_col], axis=1).reshape(P, -1)
    )
    maskdiv = np.where(
        np.eye(L, dtype=bool), np.float32(NEG_INF / SCALE), np.float32(0.0)
    ).astype(np.float32)
    eye4 = np.eye(L, dtype=np.float32)

    in_maps = []
    for i in range(num_devices):
        es_sl = slice(i * es, (i + 1) * es)
        nsl = slice(i * n_per_core, (i + 1) * n_per_core)
        m = dict(
            x=np.ascontiguousarray(x[:, nsl, :]),
            x16=np.ascontiguousarray(x16[:, nsl, :]),
            wq=pack_w(Wq, es_sl),
            wk=pack_w(Wk, es_sl),
            wv=pack_w(Wv, es_sl),
            w1=pack_w(W1, es_sl),
            w2=pack_w2(W2, es_sl),
            bqk=bqk,
            bv=bv,
            b1c=np.ascontiguousarray(b1[:, es_sl].T),
            b2c=np.ascontiguousarray(
                (b2 / num_devices).reshape(L, CH, P).transpose(2, 0, 1).reshape(P, -1)
            ),
            eye4=eye4,
            maskdiv=maskdiv,
        )
        if not trivial_affine:
            m["gamma"] = f(inputs["gamma"]).reshape(1, -1)
            m["beta"] = f(inputs["beta"]).reshape(1, -1)
        in_maps.append(m)
    return in_maps


def run_sharded(inputs: dict, trace: bool = False):
    gamma = np.asarray(inputs["gamma"], dtype=np.float32)
    beta = np.asarray(inputs["beta"], dtype=np.float32)
    trivial = bool(np.all(gamma == 1.0) and np.all(beta == 0.0))

    n_per_core = np.asarray(inputs["x"]).shape[1] // NUM_CORES
    nc = build(n_per_core, trivial)
    in_maps = make_in_maps(inputs, n_per_core, trivial)
    res = run_bass_kernel_spmd(
        nc, in_maps, core_ids=list(range(NUM_CORES)), trace=trace
    )
    out = np.concatenate(
        [np.asarray(res.results[i]["out"]) for i in range(NUM_CORES)], axis=1
    ).astype(np.float32)
    return out, res


def kernel(**inputs) -> np.ndarray:
    out, _ = run_sharded(inputs, trace=False)
    return out


# revision 48
# speedup vs baseline: 1.0782x; 1.0782x over previous
"""Trainium2 Bass kernel for CrossLevelAttention (L=4, N=65536, D=512).

Strategy (8 NeuronCores, SPMD):
  - Data-parallel shard of the node dim N (8192 nodes/core/level).
  - Pass 1 reads a host-precast bf16 copy of x (halves pass-1 HBM
    traffic; numerically identical to the previous on-chip cast):
    two DVE pairwise-add rounds quarter the PE ones-matvec work, then
    PE accumulates per-level feature sums in PSUM.
  - Warmup AllReduce at t=0 absorbs collective cold-start/skew.
  - AllReduce #1 (8 KiB) -> replicated level summaries.
  - Attention/MLP weights are E-sharded across cores (each core holds a
    64-wide output slice of Wq/Wk/Wv/W1 and the matching 64 rows of W2):
      q/k/v partial projections -> AllGather (24 KiB) -> full q,k,v
      -> scores/softmax/ctx replicated -> per-core MLP slice
      -> AllReduce #2 (8 KiB) -> replicated residual update `upd`
      broadcast across partitions via a rank-1 PE matmul.
  - Pass 2 streams fp32 x tiles [128, 8, 512] (16 KiB/partition
    descriptors); fused residual-add + LayerNorm split across DVE and
    ACT engines. Loads ride the sync queue (kept free of any
    collective-dependent instruction so prefetch fills the collective
    window); stores ride the vector queue.
"""

import os
import sys

import numpy as np

for _p in ("/opt/trn_rl_repo", "/root/.axon_site/_ro/trn_rl_repo"):
    if os.path.isdir(_p) and _p not in sys.path:
        sys.path.append(_p)

import ml_dtypes

import concourse.bass as bass
import concourse.bacc as bacc
import concourse.mybir as mybir
import concourse.tile as tile
from concourse.bass_utils import run_bass_kernel_spmd

L = 4
N_FULL = 65536
D = 512
NUM_CORES = 8
P = 128                # SBUF partitions
G1 = 16                # bf16 nodes/partition in a pass-1 tile (16 KiB/part)
G2 = 8                 # fp32 nodes/partition in a pass-2 tile (16 KiB/part)
CH = D // P            # 4 feature chunks of 128
ES = D // NUM_CORES    # 64-wide E-shard per core
NEG_INF = -1e30
SCALE = (D // 4) ** -0.5
LN_EPS = 1e-5
BB = 7                 # pass-2 tile pool depth (prefetch window)

F32 = mybir.dt.float32
BF16 = mybir.dt.bfloat16
ALU = mybir.AluOpType
ACT = mybir.ActivationFunctionType
AX = mybir.AxisListType


def build(n_per_core: int, trivial_affine: bool, num_devices: int = NUM_CORES):
    """Build the SPMD Bass program for one core."""
    assert n_per_core % (P * G1) == 0 and n_per_core % (P * G2) == 0
    T1 = n_per_core // (P * G1)        # pass-1 tiles per level
    T2 = n_per_core // (P * G2)        # pass-2 tiles per level
    n_total = n_per_core * num_devices
    rg = [list(range(num_devices))]

    nc = bacc.Bacc(
        "TRN2", target_bir_lowering=False, debug=False, num_devices=num_devices
    )

    x16_d = nc.dram_tensor("x16", [L, n_per_core, D], BF16, kind="ExternalInput")
    x_d = nc.dram_tensor("x", [L, n_per_core, D], F32, kind="ExternalInput")
    # weights pre-packed on host into the exact SBUF layouts (contiguous DMA)
    wq_d = nc.dram_tensor("wq", [P, L * CH * ES], F32, kind="ExternalInput")
    wk_d = nc.dram_tensor("wk", [P, L * CH * ES], F32, kind="ExternalInput")
    wv_d = nc.dram_tensor("wv", [P, L * CH * ES], F32, kind="ExternalInput")
    w1_d = nc.dram_tensor("w1", [P, L * CH * ES], F32, kind="ExternalInput")
    w2_d = nc.dram_tensor("w2", [ES, L * CH * P], F32, kind="ExternalInput")
    # this core's e-slice of (bq, bk, bv) -- folded in before the AllGather
    bqkv_d = nc.dram_tensor("bqkv", [ES, 3 * L], F32, kind="ExternalInput")
    b1c_d = nc.dram_tensor("b1c", [ES, L], F32, kind="ExternalInput")
    # b2/num_devices in column layout [p, (l, c)]; folded into the
    # pre-AllReduce partial so the sum over cores adds exactly b2
    b2c_d = nc.dram_tensor("b2c", [P, L * CH], F32, kind="ExternalInput")
    eye_d = nc.dram_tensor("eye4", [L, L], F32, kind="ExternalInput")
    mask_d = nc.dram_tensor("maskdiv", [L, L], F32, kind="ExternalInput")
    if not trivial_affine:
        gam_d = nc.dram_tensor("gamma", [1, L * D], F32, kind="ExternalInput")
        bet_d = nc.dram_tensor("beta", [1, L * D], F32, kind="ExternalInput")
    # bf16 output: LayerNorm-output rounding is purely multiplicative
    # (<= 2^-9 relative); the host upcasts to fp32 after gathering
    out_d = nc.dram_tensor("out", [L, n_per_core, D], BF16, kind="ExternalOutput")

    x16_r = x16_d.ap().rearrange("l (t p g) d -> l t p g d", p=P, g=G1)
    x_r = x_d.ap().rearrange("l (t p g) d -> l t p g d", p=P, g=G2)
    out_r = out_d.ap().rearrange("l (t p g) d -> l t p g d", p=P, g=G2)

    with tile.TileContext(nc) as tc:
        with (
            tc.tile_pool(name="const", bufs=1) as cpool,
            tc.tile_pool(name="wpool", bufs=1) as wpool,
            tc.tile_pool(name="xb", bufs=BB) as xpool,
            tc.tile_pool(name="ob", bufs=3) as opool,
            tc.tile_pool(name="srp", bufs=2) as srpool,
            tc.tile_pool(name="scr", bufs=2) as scrpool,
            tc.tile_pool(name="stats", bufs=4) as stpool,
            tc.tile_pool(name="small", bufs=1) as spool,
            tc.tile_pool(name="psA", bufs=1, space="PSUM") as psA,
            tc.tile_pool(name="dram", bufs=1, space="DRAM") as dram,
        ):
            ones16 = cpool.tile([P, 1], BF16)
            nc.vector.memset(ones16[:], 1.0)
            onesr = cpool.tile([1, P], F32)
            nc.vector.memset(onesr[:], 1.0)
            eps_sb = cpool.tile([P, 1], F32)
            nc.vector.memset(eps_sb[:], LN_EPS)

            # ---------------- warmup collective (absorb cold-start) ----------
            warm = spool.tile([1, 8], F32)
            nc.vector.memset(warm[:], 0.0)
            war_in = dram.tile([1, 8], F32)
            war_out = dram.tile([1, 8], F32)
            nc.gpsimd.dma_start(war_in[:], warm[:])
            nc.gpsimd.collective_compute(
                "AllReduce", ALU.add, replica_groups=rg,
                ins=[war_in.opt()], outs=[war_out.opt()],
            )

            # ---------------- small consts (tiny, ahead of the streams) ------
            bqkv_sb = cpool.tile([ES, 3, L], F32)
            b1c_sb = cpool.tile([ES, L], F32)
            b2c_sb = cpool.tile([P, L, CH], F32)
            eye_sb = cpool.tile([L, L], F32)
            mask_sb = cpool.tile([L, L], F32)
            nc.sync.dma_start(
                bqkv_sb[:], bqkv_d.ap().rearrange("e (t l) -> e t l", t=3)
            )
            nc.sync.dma_start(
                b2c_sb[:], b2c_d.ap().rearrange("p (l c) -> p l c", l=L)
            )
            for sb, dt_ in (
                (b1c_sb, b1c_d), (eye_sb, eye_d), (mask_sb, mask_d),
            ):
                nc.sync.dma_start(sb[:], dt_.ap())

            # ---------------- Pass 1: partial sums over this core's nodes ----
            psum_rows = [
                psA.tile([1, D], F32, tag=f"p{i}", name=f"prow{i}")
                for i in range(2)
            ]
            # one AllReduce per level, fired as soon as that level's sum is
            # ready -- all but the last ride under the pass-1 DMA stream
            ar1_ins = [
                dram.tile([1, D], F32, name=f"ar1i{lv}") for lv in range(L)
            ]
            ar1_outs = [
                dram.tile([1, D], F32, name=f"ar1o{lv}") for lv in range(L)
            ]
            for lv in range(L):
                pr = psum_rows[lv % 2]
                for t in range(T1):
                    xb16 = xpool.tile([P, G1, D], BF16, tag="xb", name="xb16")
                    nc.sync.dma_start(xb16[:], x16_r[lv, t])
                    # two in-place pairwise-add rounds quarter the PE work
                    nc.vector.tensor_add(
                        xb16[:, : G1 // 2, :],
                        xb16[:, : G1 // 2, :], xb16[:, G1 // 2 :, :],
                    )
                    nc.vector.tensor_add(
                        xb16[:, : G1 // 4, :],
                        xb16[:, : G1 // 4, :], xb16[:, G1 // 4 : G1 // 2, :],
                    )
                    for g in range(G1 // 4):
                        nc.tensor.matmul(
                            pr[:],
                            lhsT=ones16[:],
                            rhs=xb16[:, g, :],
                            start=(t == 0 and g == 0),
                            stop=(t == T1 - 1 and g == G1 // 4 - 1),
                        )
                srow = srpool.tile([1, D], F32, tag="srow", name="srow")
                nc.vector.tensor_scalar_mul(srow[:], pr[:], 1.0 / n_total)
                nc.gpsimd.dma_start(ar1_ins[lv][:], srow[:])
                nc.gpsimd.collective_compute(
                    "AllReduce", ALU.add, replica_groups=rg,
                    ins=[ar1_ins[lv].opt()], outs=[ar1_outs[lv].opt()],
                )

            # ---------------- weights (contiguous descriptors) ---------------
            wq_sb = wpool.tile([P, L, CH, ES], F32)
            wk_sb = wpool.tile([P, L, CH, ES], F32)
            wv_sb = wpool.tile([P, L, CH, ES], F32)
            w1_sb = wpool.tile([P, L, CH, ES], F32)
            for wsb, wd in ((wq_sb, wq_d), (wk_sb, wk_d), (wv_sb, wv_d), (w1_sb, w1_d)):
                nc.sync.dma_start(
                    wsb[:], wd.ap().rearrange("p (l c e) -> p l c e", l=L, c=CH)
                )
            w2_sb = wpool.tile([ES, L, CH, P], F32)
            nc.sync.dma_start(
                w2_sb[:], w2_d.ap().rearrange("m (l c p) -> m l c p", l=L, c=CH)
            )

            # ---------------- pass-2 prefetch (fills the collective window) --
            p2_tiles = []
            for i in range(BB):
                lv, t = divmod(i, T2)
                xb = xpool.tile([P, G2, D], F32, tag="xb", name="xbp")
                nc.sync.dma_start(xb[:], x_r[lv, t])
                p2_tiles.append(xb)

            # ------- per-level: summary readback, transpose, q/k/v -----------
            summ_col = spool.tile([P, CH, L], F32)
            psum_sc = psA.tile([P, CH, L], F32, tag="big", name="psum_sc")
            psum_qkv = psA.tile([ES, 3, L], F32, tag="h", name="psum_qkv")
            for lv in range(L):
                srow_lv = spool.tile([1, D], F32, name=f"summr{lv}")
                nc.gpsimd.dma_start(srow_lv[:], ar1_outs[lv][:])
                for c in range(CH):
                    nc.tensor.matmul(
                        psum_sc[:, c, lv : lv + 1],
                        lhsT=srow_lv[:, bass.ts(c, P)],
                        rhs=onesr[:, 0:1],
                        is_transpose=True,
                        start=True,
                        stop=True,
                    )
                nc.vector.tensor_copy(
                    summ_col[:, :, lv : lv + 1], psum_sc[:, :, lv : lv + 1]
                )
                for ti, wsb in enumerate((wq_sb, wk_sb, wv_sb)):
                    for c in range(CH):
                        nc.tensor.matmul(
                            psum_qkv[:, ti, lv : lv + 1],
                            lhsT=wsb[:, lv, c, :],
                            rhs=summ_col[:, c, lv : lv + 1],
                            start=(c == 0),
                            stop=(c == CH - 1),
                        )
            qkv_sb = spool.tile([ES, 3, L], F32)
            nc.vector.scalar_tensor_tensor(
                qkv_sb[:], psum_qkv[:], 1.0, bqkv_sb[:], ALU.mult, ALU.add
            )

            ag_in = dram.tile([ES, 3 * L], F32)
            ag_out = dram.tile([ES * num_devices, 3 * L], F32)
            nc.gpsimd.dma_start(ag_in[:], qkv_sb[:])
            nc.gpsimd.collective_compute(
                "AllGather", ALU.bypass, replica_groups=rg,
                ins=[ag_in.opt()], outs=[ag_out.opt()],
            )

            # ag_out rows = global e index (rank-major), cols = (tensor, level)
            # loads ride the scalar queue: the sync queue must stay free of
            # collective-dependent instructions so pass-2 prefetch flows.
            ag_r = ag_out[:].rearrange("(c p) (t l) -> t p c l", p=P, l=L)
            q_col = spool.tile([P, CH, L], F32)
            k_col = spool.tile([P, CH, L], F32)
            nc.scalar.dma_start(q_col[:], ag_r[0])
            nc.scalar.dma_start(k_col[:], ag_r[1])
            v_row = spool.tile([L, D], F32)
            nc.scalar.dma_start(
                v_row[:], ag_out[:].rearrange("e (t l) -> t l e", l=L)[2]
            )

            # ---------------- scores / masked softmax ------------------------
            psum_s = psA.tile([L, L], F32, tag="u", name="psum_s")
            for c in range(CH):
                nc.tensor.matmul(
                    psum_s[:],
                    lhsT=q_col[:, c, :],
                    rhs=k_col[:, c, :],
                    start=(c == 0),
                    stop=(c == CH - 1),
                )
            s_sb = spool.tile([L, L], F32)
            nc.vector.tensor_add(s_sb[:], psum_s[:], mask_sb[:])
            probs = spool.tile([L, L], F32)
            nc.scalar.activation(probs[:], s_sb[:], ACT.Exp, scale=SCALE)
            rs = spool.tile([L, 1], F32)
            nc.vector.tensor_reduce(rs[:], probs[:], axis=AX.X, op=ALU.add)
            rcp = spool.tile([L, 1], F32)
            nc.vector.reciprocal(rcp[:], rs[:])
            pn = spool.tile([L, L], F32)
            nc.vector.tensor_scalar_mul(pn[:], probs[:], rcp[:])

            psum_pT = psA.tile([L, L], F32, tag="p0", name="psum_pT")
            nc.tensor.transpose(psum_pT[:], pn[:], eye_sb[:])
            pT = spool.tile([L, L], F32)
            nc.vector.tensor_copy(pT[:], psum_pT[:])

            # ---------------- ctx (column layout), per-core MLP slice --------
            psum_ctx = psA.tile([P, CH, L], F32, tag="p1", name="psum_ctx")
            for c in range(CH):
                nc.tensor.matmul(
                    psum_ctx[:, c, :],
                    lhsT=v_row[:, bass.ts(c, P)],
                    rhs=pT[:],
                    start=(c == 0),
                    stop=(c == CH - 1),
                )
            ctx_col = spool.tile([P, CH, L], F32)
            nc.vector.tensor_copy(ctx_col[:], psum_ctx[:])

            psum_h = psA.tile([ES, L], F32, tag="h", name="psum_h")
            for lv in range(L):
                for c in range(CH):
                    nc.tensor.matmul(
                        psum_h[:, lv : lv + 1],
                        lhsT=w1_sb[:, lv, c, :],
                        rhs=ctx_col[:, c, lv : lv + 1],
                        start=(lv == 0 and c == 0),
                        stop=(lv == L - 1 and c == CH - 1),
                    )
            h_sb = spool.tile([ES, L], F32)
            nc.vector.scalar_tensor_tensor(
                h_sb[:], psum_h[:], 1.0, b1c_sb[:], ALU.mult, ALU.add
            )
            nc.vector.tensor_relu(h_sb[:], h_sb[:])

            # upd partial in column layout: upd[e=c*128+p, l]
            psum_u = psA.tile([P, L, CH], F32, tag="u", name="psum_u")
            for lv in range(L):
                for c in range(CH):
                    nc.tensor.matmul(
                        psum_u[:, lv, c : c + 1],
                        lhsT=w2_sb[:, lv, c, :],
                        rhs=h_sb[:, lv : lv + 1],
                        start=(lv == 0 and c == 0),
                        stop=(lv == L - 1 and c == CH - 1),
                    )
            up_sb = spool.tile([P, L, CH], F32)
            nc.vector.scalar_tensor_tensor(
                up_sb[:], psum_u[:], 1.0, b2c_sb[:], ALU.mult, ALU.add
            )

            ar3_in = dram.tile([P, L * CH], F32)
            ar3_out = dram.tile([P, L * CH], F32)
            nc.gpsimd.dma_start(ar3_in[:], up_sb[:])
            nc.gpsimd.collective_compute(
                "AllReduce", ALU.add, replica_groups=rg,
                ins=[ar3_in.opt()], outs=[ar3_out.opt()],
            )
            # read back on one partition: flattened free axis of us_sb is
            # the feature index d = c*128 + p per level (b2 already summed in)
            us_sb = spool.tile([1, L, CH, P], F32)
            ar3_r = ar3_out[:].rearrange("p (l c) -> l c p", l=L)
            for lv in range(L):
                nc.gpsimd.dma_start(us_sb[:, lv], ar3_r[lv])
            upd_row = us_sb[:].rearrange("o l c p -> o (l c p)")

            # broadcast upd across partitions with a rank-1 PE matmul;
            # per-level so pass-2 level 0 can start after the first copy
            psum_ubc = psA.tile([P, L, D], F32, tag="big", name="psum_ubc")
            upd_bc = cpool.tile([P, L, D], F32)
            for lv in range(L):
                nc.tensor.matmul(
                    psum_ubc[:, lv, :],
                    lhsT=onesr[:],
                    rhs=upd_row[:, bass.ts(lv, D)],
                    start=True,
                    stop=True,
                )
                nc.vector.tensor_copy(upd_bc[:, lv, :], psum_ubc[:, lv, :])

            if not trivial_affine:
                gam_row = spool.tile([1, L * D], F32)
                bet_row = spool.tile([1, L * D], F32)
                nc.scalar.dma_start(gam_row[:], gam_d.ap())
                nc.scalar.dma_start(bet_row[:], bet_d.ap())
                psum_gbc = psA.tile([P, L, D], F32, tag="big", name="psum_gbc")
                gam_bc = cpool.tile([P, L, D], F32)
                bet_bc = cpool.tile([P, L, D], F32)
                for lv in range(L):
                    nc.tensor.matmul(
                        psum_gbc[:, lv, :], lhsT=onesr[:],
                        rhs=gam_row[:, bass.ts(lv, D)], start=True, stop=True,
                    )
                nc.scalar.activation(gam_bc[:], psum_gbc[:], ACT.Identity)
                for lv in range(L):
                    nc.tensor.matmul(
                        psum_gbc[:, lv, :], lhsT=onesr[:],
                        rhs=bet_row[:, bass.ts(lv, D)], start=True, stop=True,
                    )
                nc.scalar.activation(bet_bc[:], psum_gbc[:], ACT.Identity)

            # ---------------- Pass 2: residual + LayerNorm -------------------
            NSC = 3  # normalize groups handled by the scalar engine
            idx = 0
            for lv in range(L):
                for t in range(T2):
                    if idx < BB:
                        xb = p2_tiles[idx]
                    else:
                        xb = xpool.tile([P, G2, D], F32, tag="xb", name="xb")
                        nc.sync.dma_start(xb[:], x_r[lv, t])
                    idx += 1

                    sums = stpool.tile([P, G2], F32, tag="sums")
                    ssq = stpool.tile([P, G2], F32, tag="ssq")
                    # y = x + upd (in place), per-node sum as a side effect
                    for g in range(G2):
                        nc.vector.scalar_tensor_tensor(
                            xb[:, g, :], xb[:, g, :], 1.0, upd_bc[:, lv, :],
                            ALU.mult, ALU.add, accum_out=sums[:, g : g + 1],
                        )
                    scr = scrpool.tile([P, D], F32, tag="scr")
                    for g in range(G2):
                        nc.scalar.activation(
                            scr[:], xb[:, g, :], ACT.Square,
                            accum_out=ssq[:, g : g + 1],
                        )
                    # fused stats on gpsimd/ACT keep the DVE free for the
                    # big elementwise ops
                    msq = stpool.tile([P, G2], F32, tag="msq")
                    nc.vector.scalar_tensor_tensor(
                        msq[:], sums[:], 1.0 / (D * D), sums[:],
                        ALU.mult, ALU.mult,
                    )
                    var = stpool.tile([P, G2], F32, tag="var")
                    nc.vector.scalar_tensor_tensor(
                        var[:], ssq[:], 1.0 / D, msq[:], ALU.mult, ALU.subtract
                    )
                    std = stpool.tile([P, G2], F32, tag="std")
                    nc.scalar.activation(std[:], var[:], ACT.Sqrt, bias=eps_sb[:])
                    inv = stpool.tile([P, G2], F32, tag="inv")
                    nc.vector.reciprocal(inv[:], std[:])
                    nmi = stpool.tile([P, G2], F32, tag="nmi")
                    nc.vector.scalar_tensor_tensor(
                        nmi[:], sums[:], -1.0 / D, inv[:], ALU.mult, ALU.mult
                    )

                    ob = opool.tile([P, G2, D], BF16, tag="ob")
                    if trivial_affine:
                        for g in range(G2):
                            if g < NSC:
                                nc.scalar.activation(
                                    ob[:, g, :], xb[:, g, :], ACT.Identity,
                                    bias=nmi[:, g : g + 1],
                                    scale=inv[:, g : g + 1],
                                )
                            else:
                                nc.vector.tensor_scalar(
                                    ob[:, g, :], xb[:, g, :],
                                    inv[:, g : g + 1], nmi[:, g : g + 1],
                                    ALU.mult, ALU.add,
                                )
                    else:
                        for g in range(G2):
                            if g < NSC:
                                nc.scalar.activation(
                                    xb[:, g, :], xb[:, g, :], ACT.Identity,
                                    bias=nmi[:, g : g + 1],
                                    scale=inv[:, g : g + 1],
                                )
                            else:
                                nc.vector.tensor_scalar(
                                    xb[:, g, :], xb[:, g, :],
                                    inv[:, g : g + 1], nmi[:, g : g + 1],
                                    ALU.mult, ALU.add,
                                )
                            nc.vector.tensor_mul(
                                xb[:, g, :], xb[:, g, :], gam_bc[:, lv, :]
                            )
                            nc.gpsimd.tensor_tensor(
                                ob[:, g, :], xb[:, g, :], bet_bc[:, lv, :],
                                op=ALU.add,
                            )
                    nc.gpsimd.dma_start(out_r[lv, t], ob[:])

    nc.compile()
    return nc


def make_in_maps(inputs: dict, n_per_core: int, trivial_affine: bool,
                 num_devices: int = NUM_CORES):
    """Shard full inputs into per-core input maps."""
    f = lambda a: np.ascontiguousarray(np.asarray(a, dtype=np.float32))
    x = f(inputs["x"])
    x16 = x.astype(ml_dtypes.bfloat16)
    Wq, Wk, Wv = f(inputs["Wq"]), f(inputs["Wk"]), f(inputs["Wv"])
    W1, W2 = f(inputs["W1"]), f(inputs["W2"])
    bq, bk, bv = f(inputs["bq"]), f(inputs["bk"]), f(inputs["bv"])
    b1, b2 = f(inputs["b1"]), f(inputs["b2"])

    es = D // num_devices

    def pack_w(W, es_sl):
        # [l, d, e] -> [p, (l, c, e)] with d = c*128 + p
        w = W[:, :, es_sl].reshape(L, CH, P, es)
        return np.ascontiguousarray(w.transpose(2, 0, 1, 3).reshape(P, -1))

    def pack_w2(W, es_sl):
        # [l, e, d] -> [e, (l, c, p)] with d = c*128 + p
        w = W[:, es_sl, :].reshape(L, es, CH, P)
        return np.ascontiguousarray(w.transpose(1, 0, 2, 3).reshape(es, -1))

    # bias in column layout [p, c, l] with d = c*128 + p; q and k packed
    bq_col = bq.reshape(L, CH, P).transpose(2, 1, 0)
    bk_col = bk.reshape(L, CH, P).transpose(2, 1, 0)
    bqk = np.ascontiguousarray(
        np.stack([bq_col, bk_col], axis=1).reshape(P, -1)
    )
    maskdiv = np.where(
        np.eye(L, dtype=bool), np.float32(NEG_INF / SCALE), np.float32(0.0)
    ).astype(np.float32)
    eye4 = np.eye(L, dtype=np.float32)

    in_maps = []
    for i in range(num_devices):
        es_sl = slice(i * es, (i + 1) * es)
        nsl = slice(i * n_per_core, (i + 1) * n_per_core)
        m = dict(
            x=np.ascontiguousarray(x[:, nsl, :]),
            x16=np.ascontiguousarray(x16[:, nsl, :]),
            wq=pack_w(Wq, es_sl),
            wk=pack_w(Wk, es_sl),
            wv=pack_w(Wv, es_sl),
            w1=pack_w(W1, es_sl),
            w2=pack_w2(W2, es_sl),
            bqkv=np.ascontiguousarray(
                np.stack([bq, bk, bv], axis=0)[:, :, es_sl]
                .transpose(2, 0, 1).reshape(es, -1)
            ),
            b1c=np.ascontiguousarray(b1[:, es_sl].T),
            b2c=np.ascontiguousarray(
                (b2 / num_devices).reshape(L, CH, P).transpose(2, 0, 1).reshape(P, -1)
            ),
            eye4=eye4,
            maskdiv=maskdiv,
        )
        if not trivial_affine:
            m["gamma"] = f(inputs["gamma"]).reshape(1, -1)
            m["beta"] = f(inputs["beta"]).reshape(1, -1)
        in_maps.append(m)
    return in_maps


def run_sharded(inputs: dict, trace: bool = False):
    gamma = np.asarray(inputs["gamma"], dtype=np.float32)
    beta = np.asarray(inputs["beta"], dtype=np.float32)
    trivial = bool(np.all(gamma == 1.0) and np.all(beta == 0.0))

    n_per_core = np.asarray(inputs["x"]).shape[1] // NUM_CORES
    nc = build(n_per_core, trivial)
    in_maps = make_in_maps(inputs, n_per_core, trivial)
    res = run_bass_kernel_spmd(
        nc, in_maps, core_ids=list(range(NUM_CORES)), trace=trace
    )
    out = np.concatenate(
        [np.asarray(res.results[i]["out"]) for i in range(NUM_CORES)], axis=1
    ).astype(np.float32)
    return out, res


def kernel(**inputs) -> np.ndarray:
    out, _ = run_sharded(inputs, trace=False)
    return out
